# revision 1
# baseline (speedup 1.0000x reference)
"""Causal self-attention (B=2, N=2048, D=2048, H=16, hd=128) on 8 Trainium2
NeuronCores.

Strategy (tensor-parallel over heads, 2 heads/core):
  - Host: transpose x / weights, build RoPE tables + triangular mask consts,
    slice w_qkv rows per head-group.
  - Device, per core (same SPMD program, different input data):
    Phase A: qkvT projection (f32r matmuls, outputs in [d, n] layout) + RoPE
             (partition-rotate via SBUF DMA + DVE mul/add).
    Phase B: attention fully in transposed orientation: S.T = kT.T @ qT
             (PE), P.T = exp(S.T) (ACT), causal mask via sliced triangular
             const (DVE), O.T accumulated as vT.T @ P.T (PE, PSUM accum).
             Softmax denominators: DVE-accumulate P.T tiles, ones-matmul to
             reduce over partitions, reciprocal, ones-bcast matmul, scale.
    AllToAll: reshard O.T from head-sharded to row-sharded (full inner dim).
    Phase C: o_proj on the 512-row shard: out = O.T_full.T @ w_o.T.
  - Host: concatenate the 8 row-shards.

Zero on-device transposes: every matmul consumes operands in the layout the
previous phase produced.
"""

import sys
import time

import ml_dtypes
import numpy as np

sys.path.insert(0, "/opt/trn_rl_repo")

import concourse.bacc as bacc  # noqa: E402
import concourse.bass as bass  # noqa: E402
import concourse.mybir as mybir  # noqa: E402
import concourse.tile as tile  # noqa: E402
from concourse import bass_utils  # noqa: E402

F32 = mybir.dt.float32
BF16 = mybir.dt.bfloat16

B, N, D = 2, 2048, 2048
H, HD = 16, 128
NC = 8
HPC = H // NC          # heads per core
BN = B * N             # 4096
NSH = BN // NC         # output rows per core
INNER = H * HD
ROPE_BASE = 10000.0

_CACHE = {}

LAST_EXEC_NS = None
LAST_RESULTS = None


def _build_program():
    nc = bacc.Bacc(
        "TRN2",
        target_bir_lowering=False,
        debug=False,
        enable_asserts=False,
        num_devices=NC,
    )
    xT = nc.dram_tensor("xT", [D, BN], BF16, kind="ExternalInput").ap()
    wqkT = nc.dram_tensor("wqkT", [D, 4 * HD], BF16, kind="ExternalInput").ap()
    wvT = nc.dram_tensor("wvT", [D, HPC * HD], BF16, kind="ExternalInput").ap()
    woT = nc.dram_tensor("woT", [INNER, D], BF16, kind="ExternalInput").ap()
    tabs = nc.dram_tensor("tabs", [4, HD, BN], BF16, kind="ExternalInput").ap()
    tri = nc.dram_tensor("tri", [128, 1024], BF16, kind="ExternalInput").ap()
    out = nc.dram_tensor("out", [NSH, D], F32, kind="ExternalOutput").ap()
    a2a_in = nc.dram_tensor("a2a_in", [NC, HPC, 128, 512], BF16).ap()
    a2a_out = nc.dram_tensor("a2a_out", [NC, HPC, 128, 512], BF16).ap()

    MUL = mybir.AluOpType.mult
    ADD = mybir.AluOpType.add
    SUB = mybir.AluOpType.subtract
    EXP = mybir.ActivationFunctionType.Exp

    with tile.TileContext(nc, num_cores=NC) as tc:
        with (
            tc.tile_pool(name="const", bufs=1) as constp,
            tc.tile_pool(name="wqk", bufs=1) as wqkp,
            tc.tile_pool(name="wv", bufs=1) as wvp,
            tc.tile_pool(name="persist", bufs=1) as persist,
        ):
            tri_sb = constp.tile([128, 1024], BF16, name="tri_sb")
            nc.sync.dma_start(out=tri_sb[:, :], in_=tri[:, :])
            ones_col = constp.tile([128, 1], F32, name="ones_col")
            nc.vector.memset(ones_col[:, :], 1.0)
            ones_row = constp.tile([1, 128], F32, name="ones_row")
            nc.vector.memset(ones_row[:, :], 1.0)
            wqk_sb = wqkp.tile([128, 16, 512], BF16, name="wqk_sb")
            nc.sync.dma_start(
                out=wqk_sb[:, :, :],
                in_=wqkT.rearrange("(k p) m -> p k m", p=128),
            )
            wv_sb = wvp.tile([128, 16, 256], BF16, name="wv_sb")
            nc.sync.dma_start(
                out=wv_sb[:, :, :],
                in_=wvT.rearrange("(k p) m -> p k m", p=128),
            )

            with (
                tc.tile_pool(name="xt", bufs=3) as xtp,
                tc.tile_pool(name="tab", bufs=2) as tabp,
                tc.tile_pool(name="rope", bufs=2) as ropep,
                tc.tile_pool(name="pt", bufs=3) as ptp,
                tc.tile_pool(name="rs", bufs=2) as rsp,
                tc.tile_pool(name="small", bufs=2) as smallp,
                tc.tile_pool(name="ots", bufs=2) as otsp,
                tc.tile_pool(name="pst", bufs=3, space="PSUM") as pstp,
                tc.tile_pool(name="pov", bufs=3, space="PSUM") as povp,
                tc.tile_pool(name="psmall", bufs=1, space="PSUM") as psmallp,
            ):
                for b in range(B):
                    qkT_sb = persist.tile(
                        [128, 4, N], BF16, tag="qkT", name=f"qkT_b{b}"
                    )
                    vT_sb = persist.tile(
                        [128, 16, HPC * HD], BF16, tag="vT", name=f"vT_b{b}"
                    )
                    # ---------------- phase A: projection + RoPE ----------
                    for j in range(4):
                        n0 = b * N + 512 * j
                        xh = []
                        for half in range(2):
                            t = xtp.tile(
                                [128, 8, 512], BF16, tag="xt", name=f"xt_{b}_{j}_{half}"
                            )
                            nc.sync.dma_start(
                                out=t[:, :, :],
                                in_=xT.rearrange("(k p) n -> p k n", p=128)[
                                    :, 8 * half : 8 * half + 8, n0 : n0 + 512
                                ],
                            )
                            xh.append(t)
                        tabt = []
                        for ti in range(4):
                            tt = tabp.tile([128, 512], BF16, tag=f"tab{ti}", name=f"tab{ti}_{b}_{j}")
                            nc.sync.dma_start(out=tt[:, :], in_=tabs[ti, :, n0 : n0 + 512])
                            tabt.append(tt)
                        for pair in (0, 2):
                            psA = pstp.tile([128, 512], F32, tag="pst", name=f"psA_{b}_{j}_{pair}")
                            psB = pstp.tile([128, 512], F32, tag="pst", name=f"psB_{b}_{j}_{pair}")
                            for mt, pst_ in ((pair, psA), (pair + 1, psB)):
                                for k in range(16):
                                    nc.tensor.matmul(
                                        pst_[:, :],
                                        lhsT=(wqk_sb[:, k, 128 * mt : 128 * mt + 128]),
                                        rhs=(xh[k // 8][:, k % 8, :]),
                                        start=(k == 0),
                                        stop=(k == 15),
                                    )
                            ci = 0 if pair == 0 else 2
                            t1 = ropep.tile([128, 512], BF16, tag="t1", name=f"t1_{b}_{j}_{pair}")
                            t2 = ropep.tile([128, 512], BF16, tag="t2", name=f"t2_{b}_{j}_{pair}")
                            t3 = ropep.tile([128, 512], BF16, tag="t3", name=f"t3_{b}_{j}_{pair}")
                            t4 = ropep.tile([128, 512], BF16, tag="t4", name=f"t4_{b}_{j}_{pair}")
                            nc.vector.tensor_tensor(t1[:, :], psA[:, :], tabt[ci][:, :], MUL)
                            nc.vector.tensor_tensor(t2[:, :], psB[:, :], tabt[ci + 1][:, :], MUL)
                            nc.vector.tensor_tensor(t3[:, :], psB[:, :], tabt[ci][:, :], MUL)
                            nc.vector.tensor_tensor(t4[:, :], psA[:, :], tabt[ci + 1][:, :], MUL)
                            nc.vector.tensor_tensor(
                                qkT_sb[:, pair, 512 * j : 512 * (j + 1)], t1[:, :], t2[:, :], SUB
                            )
                            nc.vector.tensor_tensor(
                                qkT_sb[:, pair + 1, 512 * j : 512 * (j + 1)], t3[:, :], t4[:, :], ADD
                            )
                        for mt in range(4):
                            pv = povp.tile([128, 256], F32, tag="pov", name=f"psV_{b}_{j}_{mt}")
                            for k in range(16):
                                nc.tensor.matmul(
                                    pv[:, :],
                                    lhsT=(xh[k // 8][:, k % 8, 128 * mt : 128 * mt + 128]),
                                    rhs=(wv_sb[:, k, :]),
                                    start=(k == 0),
                                    stop=(k == 15),
                                )
                            nc.scalar.copy(vT_sb[:, 4 * j + mt, :], pv[:, :])
                    # ---------------- phase B: attention ------------------
                    for h in range(HPC):
                        for j in range(4):
                            ov = povp.tile([128, 512], F32, tag="pov", name=f"ov_{b}_{h}_{j}")
                            rs_c = rsp.tile([128, 512], F32, tag="rs", name=f"rs_{b}_{h}_{j}")
                            nc.vector.memset(rs_c[:, :], 0.0)
                            for t in range(4 * j + 4):
                                st = pstp.tile(
                                    [128, 512], F32, tag="pst", name=f"st_{b}_{h}_{j}_{t}"
                                )
                                nc.tensor.matmul(
                                    st[:, :],
                                    lhsT=(qkT_sb[64 * h : 64 * h + 64, 2, 128 * t : 128 * t + 128]),
                                    rhs=(qkT_sb[64 * h : 64 * h + 64, 0, 512 * j : 512 * (j + 1)]),
                                    start=True,
                                    stop=False,
                                )
                                nc.tensor.matmul(
                                    st[:, :],
                                    lhsT=(qkT_sb[64 * h : 64 * h + 64, 3, 128 * t : 128 * t + 128]),
                                    rhs=(qkT_sb[64 * h : 64 * h + 64, 1, 512 * j : 512 * (j + 1)]),
                                    start=False,
                                    stop=True,
                                )
                                pt = ptp.tile(
                                    [128, 512], BF16, tag="pt", name=f"pt_{b}_{h}_{j}_{t}"
                                )
                                nc.scalar.activation(pt[:, :], st[:, :], EXP)
                                if t // 4 == j:
                                    f0 = 128 * t - 512 * j
                                    nc.vector.tensor_tensor(
                                        pt[:, :], pt[:, :],
                                        tri_sb[:, 512 - f0 : 1024 - f0], MUL,
                                    )
                                nc.vector.tensor_tensor(rs_c[:, :], rs_c[:, :], pt[:, :], ADD)
                                nc.tensor.matmul(
                                    ov[:, :],
                                    lhsT=(vT_sb[:, t, 128 * h : 128 * h + 128]),
                                    rhs=(pt[:, :]),
                                    start=(t == 0),
                                    stop=(t == 4 * j + 3),
                                )
                            rsum = psmallp.tile([1, 512], F32, tag="rsum", name=f"rsum_{b}_{h}_{j}")
                            nc.tensor.matmul(
                                rsum[:, :], lhsT=ones_col[:, :], rhs=rs_c[:, :],
                                start=True, stop=True,
                            )
                            rinv = smallp.tile([1, 512], F32, tag="rinv", name=f"rinv_{b}_{h}_{j}")
                            nc.vector.reciprocal(rinv[:, :], rsum[:, :])
                            binv = psmallp.tile([128, 512], F32, tag="binv", name=f"binv_{b}_{h}_{j}")
                            nc.tensor.matmul(
                                binv[:, :], lhsT=ones_row[:, :], rhs=rinv[:, :],
                                start=True, stop=True,
                            )
                            binv_sb = smallp.tile(
                                [128, 512], F32, tag="binv_sb", name=f"binvs_{b}_{h}_{j}"
                            )
                            nc.scalar.copy(binv_sb[:, :], binv[:, :])
                            ot = otsp.tile([128, 512], BF16, tag="ot", name=f"ot_{b}_{h}_{j}")
                            nc.vector.tensor_tensor(ot[:, :], ov[:, :], binv_sb[:, :], MUL)
                            nc.sync.dma_start(
                                out=a2a_in[4 * b + j, h, :, :], in_=ot[:, :]
                            )

            # ---------------- AllToAll reshard ----------------------------
            nc.gpsimd.collective_compute(
                "AllToAll",
                mybir.AluOpType.bypass,
                replica_groups=[list(range(NC))],
                ins=[a2a_in.opt()],
                outs=[a2a_out.opt()],
            )

            # ---------------- phase C: o_proj ------------------------------
            with (
                tc.tile_pool(name="opin", bufs=1) as opinp,
                tc.tile_pool(name="wo", bufs=4) as wop,
                tc.tile_pool(name="outs", bufs=4) as outsp,
                tc.tile_pool(name="pc", bufs=4, space="PSUM") as pcp,
            ):
                opin = opinp.tile([128, 16, 512], BF16, name="opin")
                nc.sync.dma_start(
                    out=opin[:, :, :],
                    in_=a2a_out.rearrange("r h p n -> p (r h) n"),
                )
                for dc in range(4):
                    pcs = [
                        pcp.tile([128, 512], F32, tag="pc", name=f"pc_{dc}_{ns}")
                        for ns in range(4)
                    ]
                    for k in range(16):
                        wo_t = wop.tile([128, 512], BF16, tag="wo", name=f"wo_{dc}_{k}")
                        nc.sync.dma_start(
                            out=wo_t[:, :],
                            in_=woT.rearrange("(k p) d -> p k d", p=128)[
                                :, k, 512 * dc : 512 * (dc + 1)
                            ],
                        )
                        for ns in range(4):
                            nc.tensor.matmul(
                                pcs[ns][:, :],
                                lhsT=(opin[:, k, 128 * ns : 128 * ns + 128]),
                                rhs=(wo_t[:, :]),
                                start=(k == 0),
                                stop=(k == 15),
                            )
                    for ns in range(4):
                        ost = outsp.tile([128, 512], F32, tag="outs", name=f"os_{dc}_{ns}")
                        nc.scalar.copy(ost[:, :], pcs[ns][:, :])
                        nc.sync.dma_start(
                            out=out[128 * ns : 128 * (ns + 1), 512 * dc : 512 * (dc + 1)],
                            in_=ost[:, :],
                        )
    nc.compile()
    return nc


def _host_prep(x, w_qkv, w_o):
    bf = ml_dtypes.bfloat16
    xT = np.ascontiguousarray(x.reshape(BN, D).T).astype(bf)
    woT = np.ascontiguousarray(np.asarray(w_o).T).astype(bf)

    inv_freq = 1.0 / (ROPE_BASE ** (np.arange(0, HD, 2, dtype=np.float32) / HD))
    ang = np.arange(N, dtype=np.float32)[:, None] * inv_freq[None, :]
    cos_h = np.cos(ang).T.astype(np.float32)      # [64, N]
    sin_h = np.sin(ang).T.astype(np.float32)      # [64, N] (magnitude)
    # duplicated for the two heads packed per 128-row block
    cos2 = np.concatenate([cos_h, cos_h], axis=0)  # [128, N]
    sin2 = np.concatenate([sin_h, sin_h], axis=0)
    cos_f = np.tile(cos2, (1, B))
    sin_f = np.tile(sin2, (1, B))
    scale = np.float32(1.0 / np.sqrt(HD))
    tabs = np.ascontiguousarray(
        np.stack([cos_f * scale, sin_f * scale, cos_f, sin_f], axis=0)
    ).astype(bf)

    p = np.arange(128)[:, None]
    c = np.arange(1024)[None, :]
    tri = (p <= c - 512).astype(bf)

    in_maps = []
    for core in range(NC):
        h0 = core * HPC
        rq = slice(h0 * HD, (h0 + HPC) * HD)
        rk = slice(INNER + h0 * HD, INNER + (h0 + HPC) * HD)
        rv = slice(2 * INNER + h0 * HD, 2 * INNER + (h0 + HPC) * HD)
        wq = w_qkv[rq].reshape(HPC, HD, D)
        wk = w_qkv[rk].reshape(HPC, HD, D)
        # row order per block: [h0_lo, h1_lo | h0_hi, h1_hi] for q then k
        wqkT = np.ascontiguousarray(
            np.concatenate(
                [wq[0, :64], wq[1, :64], wq[0, 64:], wq[1, 64:],
                 wk[0, :64], wk[1, :64], wk[0, 64:], wk[1, 64:]], axis=0
            ).T
        ).astype(bf)
        wvT = np.ascontiguousarray(w_qkv[rv].T).astype(bf)
        in_maps.append(
            dict(xT=xT, wqkT=wqkT, wvT=wvT, woT=woT, tabs=tabs, tri=tri)
        )
    return in_maps


def kernel(x, w_qkv, w_o, n_heads=None, head_dim=None, trace=False):
    global LAST_EXEC_NS, LAST_RESULTS
    x = np.asarray(x, dtype=np.float32)
    w_qkv = np.asarray(w_qkv, dtype=np.float32)
    w_o = np.asarray(w_o, dtype=np.float32)

    if "nc" not in _CACHE:
        _CACHE["nc"] = _build_program()
    nc = _CACHE["nc"]

    in_maps = _host_prep(x, w_qkv, w_o)
    res = None
    last_exc = None
    for attempt in range(4):
        try:
            res = bass_utils.run_bass_kernel_spmd(
                nc, in_maps, core_ids=list(range(NC)), trace=trace
            )
            break
        except Exception as e:  # transient compile_and_load / exec flakiness
            last_exc = e
            print(f"kernel attempt {attempt} failed: {e}", file=sys.stderr)
            time.sleep(5)
    if res is None:
        raise last_exc
    LAST_EXEC_NS = res.exec_time_ns
    LAST_RESULTS = res
    shards = [res.results[c]["out"] for c in range(NC)]
    full = np.concatenate(shards, axis=0).reshape(B, N, D).astype(np.float32)
    return full



# revision 2
# speedup vs baseline: 1.0239x; 1.0239x over previous
"""Causal self-attention (B=2, N=2048, D=2048, H=16, hd=128) on 8 Trainium2
NeuronCores — v3.

Tensor-parallel over heads (2 heads/core). v3 = v2 + full-K scores:
  - Per-head weight layout: each 128-row block of wqkT is one head's full
    head_dim, so q/k land on 128 partitions per head and the score matmul
    contracts K=128 in a single instruction (v1/v2 used two K=64 halves,
    which also kept the PE HAM throttled at 1.2 GHz through phase B).
  - RoPE rotate-half done with a PE permutation matmul (P constant) on the
    bf16 copy of the raw projection, then two DVE multiplies + one add.
  - Wide 2-bank exp, PE-accumulated softmax denominators,
    reciprocal_approx_fast, causal narrowing, pipelined score emission,
    double-buffered persist tiles, early woT prefetch (all from v2).
"""

import sys
import time

import ml_dtypes
import numpy as np

sys.path.insert(0, "/opt/trn_rl_repo")

import concourse.bacc as bacc  # noqa: E402
import concourse.bass as bass  # noqa: E402
import concourse.mybir as mybir  # noqa: E402
import concourse.tile as tile  # noqa: E402
from concourse import bass_utils  # noqa: E402

F32 = mybir.dt.float32
BF16 = mybir.dt.bfloat16

B, N, D = 2, 2048, 2048
H, HD = 16, 128
NC = 8
HPC = H // NC          # heads per core
BN = B * N             # 4096
NSH = BN // NC         # output rows per core
INNER = H * HD
ROPE_BASE = 10000.0

_CACHE = {}

LAST_EXEC_NS = None
LAST_RESULTS = None


def _build_program():
    nc = bacc.Bacc(
        "TRN2",
        target_bir_lowering=False,
        debug=False,
        enable_asserts=False,
        num_devices=NC,
    )
    xT = nc.dram_tensor("xT", [D, BN], BF16, kind="ExternalInput").ap()
    wqkT = nc.dram_tensor("wqkT", [D, 4 * HD], BF16, kind="ExternalInput").ap()
    wvT = nc.dram_tensor("wvT", [D, HPC * HD], BF16, kind="ExternalInput").ap()
    woT = nc.dram_tensor("woT", [INNER, D], BF16, kind="ExternalInput").ap()
    tabs = nc.dram_tensor("tabs", [4, HD, BN], BF16, kind="ExternalInput").ap()
    tri = nc.dram_tensor("tri", [128, 128], BF16, kind="ExternalInput").ap()
    rotp = nc.dram_tensor("rotp", [128, 128], BF16, kind="ExternalInput").ap()
    out = nc.dram_tensor("out", [NSH, D], F32, kind="ExternalOutput").ap()
    a2a_in = [
        nc.dram_tensor(f"a2a_in{h}", [NC, 128, 512], BF16).ap()
        for h in range(HPC)
    ]
    a2a_out = [
        nc.dram_tensor(f"a2a_out{h}", [NC, 128, 512], BF16).ap()
        for h in range(HPC)
    ]

    MUL = mybir.AluOpType.mult
    ADD = mybir.AluOpType.add
    SUB = mybir.AluOpType.subtract
    EXP = mybir.ActivationFunctionType.Exp

    with tile.TileContext(nc, num_cores=NC) as tc:
        with (
            tc.tile_pool(name="const", bufs=1) as constp,
            tc.tile_pool(name="wqk", bufs=1) as wqkp,
            tc.tile_pool(name="wv", bufs=1) as wvp,
            tc.tile_pool(name="persist", bufs=2) as persist,
            tc.tile_pool(name="wo", bufs=2) as wop,
        ):
            wqk_sb = wqkp.tile([128, 16, 512], BF16, name="wqk_sb")
            for kc in range(4):
                nc.sync.dma_start(
                    out=wqk_sb[:, 4 * kc : 4 * kc + 4, :],
                    in_=wqkT.rearrange("(k p) m -> p k m", p=128)[
                        :, 4 * kc : 4 * kc + 4, :
                    ],
                )
            tri_sb = constp.tile([128, 128], BF16, name="tri_sb")
            nc.sync.dma_start(out=tri_sb[:, :], in_=tri[:, :])
            rot_sb = constp.tile([128, 128], BF16, name="rot_sb")
            nc.sync.dma_start(out=rot_sb[:, :], in_=rotp[:, :])
            ones_col = constp.tile([128, 1], BF16, name="ones_col")
            nc.vector.memset(ones_col[:, :], 1.0)
            ones_row = constp.tile([1, 128], F32, name="ones_row")
            nc.vector.memset(ones_row[:, :], 1.0)
            wv_sb = wvp.tile([128, 16, 256], BF16, name="wv_sb")
            nc.sync.dma_start(
                out=wv_sb[:, :, :],
                in_=wvT.rearrange("(k p) m -> p k m", p=128),
            )
            wo_tiles = {}

            def emit_wo_prefetch(dc):
                wt = wop.tile([128, 16, 512], BF16, tag="wo2", name=f"wo2_{dc}")
                nc.sync.dma_start(
                    out=wt[:, :, :],
                    in_=woT.rearrange("(k p) d -> p k d", p=128)[
                        :, :, 512 * dc : 512 * (dc + 1)
                    ],
                )
                wo_tiles[dc] = wt

            with (
                tc.tile_pool(name="xt", bufs=3) as xtp,
                tc.tile_pool(name="tab", bufs=2) as tabp,
                tc.tile_pool(name="rope", bufs=2) as ropep,
                tc.tile_pool(name="pt", bufs=3) as ptp,
                tc.tile_pool(name="small", bufs=2) as smallp,
                tc.tile_pool(name="ots", bufs=2) as otsp,
                tc.tile_pool(name="pq", bufs=2, space="PSUM") as pqp,
                tc.tile_pool(name="pov", bufs=2, space="PSUM") as povp,
                tc.tile_pool(name="psmall", bufs=1, space="PSUM") as psmallp,
            ):
                for b in range(B):
                    qkT_sb = persist.tile(
                        [128, 4, N], BF16, tag="qkT", name=f"qkT_b{b}"
                    )
                    vT_sb = persist.tile(
                        [128, 16, HPC * HD], BF16, tag="vT", name=f"vT_b{b}"
                    )
                    # ---------------- phase A: projection + RoPE ----------
                    for j in range(4):
                        n0 = b * N + 512 * j
                        xh = []
                        for half in range(2):
                            t = xtp.tile(
                                [128, 8, 512], BF16, tag="xt",
                                name=f"xt_{b}_{j}_{half}",
                            )
                            for q4 in range(2):
                                nc.sync.dma_start(
                                    out=t[:, 4 * q4 : 4 * q4 + 4, :],
                                    in_=xT.rearrange("(k p) n -> p k n", p=128)[
                                        :,
                                        8 * half + 4 * q4 : 8 * half + 4 * q4 + 4,
                                        n0 : n0 + 512,
                                    ],
                                )
                            xh.append(t)
                        tab4 = tabp.tile(
                            [128, 4, 512], BF16, tag="tab", name=f"tab_{b}_{j}"
                        )
                        nc.sync.dma_start(
                            out=tab4[:, :, :],
                            in_=tabs.rearrange("t p n -> p t n")[
                                :, :, n0 : n0 + 512
                            ],
                        )
                        # qk2[:,0,:] = raw projection (PSUM); qk2[:,1,:] = rotated
                        # (PE permutation of the bf16 copy).  One pending rope
                        # per projection block keeps PE dense.
                        pending_rope = []

                        def emit_rope(mt, qk2):
                            # ci: 0/1 = scaled cos/sin (q heads), 2/3 = cos/sin (k)
                            ci = 0 if mt < 2 else 2
                            raw = ropep.tile(
                                [128, 512], BF16, tag="raw", name=f"raw_{b}_{j}_{mt}"
                            )
                            nc.scalar.copy(raw[:, :], qk2[:, 0, :])
                            nc.tensor.matmul(
                                qk2[:, 1, :], lhsT=rot_sb[:, :], rhs=raw[:, :],
                                start=True, stop=True,
                            )
                            t1 = ropep.tile([128, 512], BF16, tag="t1", name=f"t1_{b}_{j}_{mt}")
                            t2 = ropep.tile([128, 512], BF16, tag="t2", name=f"t2_{b}_{j}_{mt}")
                            nc.vector.tensor_tensor(t1[:, :], raw[:, :], tab4[:, ci, :], MUL)
                            nc.vector.tensor_tensor(t2[:, :], qk2[:, 1, :], tab4[:, ci + 1, :], MUL)
                            nc.vector.tensor_tensor(
                                qkT_sb[:, mt, 512 * j : 512 * (j + 1)], t1[:, :], t2[:, :], ADD
                            )

                        for mt in range(4):
                            qk2 = pqp.tile(
                                [128, 2, 512], F32, tag="st2",
                                name=f"qk2_{b}_{j}_{mt}",
                            )
                            for k in range(16):
                                nc.tensor.matmul(
                                    qk2[:, 0, :],
                                    lhsT=(wqk_sb[:, k, 128 * mt : 128 * mt + 128]),
                                    rhs=(xh[k // 8][:, k % 8, :]),
                                    start=(k == 0),
                                    stop=(k == 15),
                                )
                            if pending_rope:
                                emit_rope(*pending_rope.pop(0))
                            pending_rope.append((mt, qk2))
                        for mt in range(4):
                            pv = povp.tile(
                                [128, 256], F32, tag="ov", name=f"psV_{b}_{j}_{mt}"
                            )
                            for k in range(16):
                                nc.tensor.matmul(
                                    pv[:, :],
                                    lhsT=(xh[k // 8][:, k % 8, 128 * mt : 128 * mt + 128]),
                                    rhs=(wv_sb[:, k, :]),
                                    start=(k == 0),
                                    stop=(k == 15),
                                )
                            if pending_rope:
                                emit_rope(*pending_rope.pop(0))
                            nc.vector.tensor_copy(vT_sb[:, 4 * j + mt, :], pv[:, :])
                        while pending_rope:
                            emit_rope(*pending_rope.pop(0))
                    # ---------------- phase B: attention ------------------
                    if b == 0:
                        emit_wo_prefetch(0)
                        emit_wo_prefetch(1)
                    st2_store = {}
                    unit_acc = {}

                    def emit_scores(h, j, tt):
                        st2 = pqp.tile(
                            [128, 2, 512], F32, tag="st2",
                            name=f"st2_{b}_{h}_{j}_{tt}",
                        )
                        f0s = []
                        for u in (0, 1):
                            t = tt + u
                            f0 = max(0, 128 * t - 512 * j)
                            f0s.append(f0)
                            nc.tensor.matmul(
                                st2[:, u, f0:512],
                                lhsT=(qkT_sb[:, 2 + h, 128 * t : 128 * t + 128]),
                                rhs=(qkT_sb[:, h, 512 * j + f0 : 512 * (j + 1)]),
                                start=True,
                                stop=True,
                            )
                        st2_store[(h, j, tt)] = (st2, f0s)

                    def emit_consume(h, j, tt):
                        nt = 4 * j + 4
                        if tt == 0:
                            unit_acc[(h, j)] = (
                                povp.tile([128, 512], F32, tag="ov", name=f"ov_{b}_{h}_{j}"),
                                ropep.tile([128, 512], BF16, tag="rsc", name=f"rsc_{b}_{h}_{j}"),
                            )
                        ov, rs_c = unit_acc[(h, j)]
                        st2, f0s = st2_store.pop((h, j, tt))
                        ws = min(f0s)
                        pt2 = ptp.tile(
                            [128, 2, 512], BF16, tag="pt", name=f"pt_{b}_{h}_{j}_{tt}"
                        )
                        nc.scalar.activation(
                            pt2[:, :, ws:512], st2[:, :, ws:512], EXP
                        )
                        for u in (0, 1):
                            t = tt + u
                            if t // 4 == j:
                                f0 = f0s[u]
                                nc.vector.tensor_tensor(
                                    pt2[:, u, f0 : f0 + 128],
                                    pt2[:, u, f0 : f0 + 128],
                                    tri_sb[:, :],
                                    MUL,
                                )
                        for u in (0, 1):
                            t = tt + u
                            f0 = f0s[u]
                            nc.tensor.matmul(
                                ov[:, f0:512],
                                lhsT=(vT_sb[:, t, 128 * h : 128 * h + 128]),
                                rhs=(pt2[:, u, f0:512]),
                                start=(t == 0),
                                stop=(t == nt - 1),
                            )
                            if t == 0:
                                nc.vector.tensor_copy(rs_c[:, :], pt2[:, 0, :])
                            else:
                                nc.vector.tensor_tensor(
                                    rs_c[:, f0:512], rs_c[:, f0:512],
                                    pt2[:, u, f0:512], ADD,
                                )

                    def make_finalize(h, j):
                        ov, rs_c = unit_acc.pop((h, j))

                        def fin():
                            rsum = psmallp.tile([1, 512], F32, tag="rsum", name=f"rsum_{b}_{h}_{j}")
                            nc.tensor.matmul(
                                rsum[:, :], lhsT=ones_col[:, :], rhs=rs_c[:, :],
                                start=True, stop=True,
                            )
                            rsum_sb = smallp.tile([1, 512], F32, tag="rsum_sb", name=f"rsb_{b}_{h}_{j}")
                            nc.scalar.copy(rsum_sb[:, :], rsum[:, :])
                            binv = psmallp.tile([128, 512], F32, tag="binv", name=f"binv_{b}_{h}_{j}")
                            nc.tensor.matmul(
                                binv[:, :], lhsT=ones_row[:, :], rhs=rsum_sb[:, :],
                                start=True, stop=True,
                            )
                            rb = smallp.tile([128, 512], F32, tag="rb", name=f"rb_{b}_{h}_{j}")
                            nc.vector.reciprocal_approx_fast(rb[:, :], binv[:, :])
                            ot = otsp.tile([128, 512], BF16, tag="ot", name=f"ot_{b}_{h}_{j}")
                            nc.vector.tensor_tensor(ot[:, :], ov[:, :], rb[:, :], MUL)
                            nc.sync.dma_start(
                                out=a2a_in[h][4 * b + j, :, :], in_=ot[:, :]
                            )
                        return fin

                    all_groups = []
                    for h in range(HPC):
                        for j in range(4):
                            for tt in range(0, 4 * j + 4, 2):
                                all_groups.append((h, j, tt))
                    pending_fin = None
                    emit_scores(*all_groups[0])
                    for gi, (h, j, tt) in enumerate(all_groups):
                        if gi + 1 < len(all_groups):
                            emit_scores(*all_groups[gi + 1])
                        if tt == 0 and pending_fin is not None:
                            pending_fin()
                            pending_fin = None
                        if b == 1 and (h, j, tt) == (1, 0, 0):
                            # every h=0 finalize (both batches) has been emitted;
                            # reshard head 0 while head 1 attention still runs
                            nc.gpsimd.collective_compute(
                                "AllToAll",
                                mybir.AluOpType.bypass,
                                replica_groups=[list(range(NC))],
                                ins=[a2a_in[0].opt()],
                                outs=[a2a_out[0].opt()],
                            )
                        emit_consume(h, j, tt)
                        if tt == 4 * j + 2:  # last group of unit
                            pending_fin = make_finalize(h, j)
                    pending_fin()
                    pending_fin = None

            # ---------------- AllToAll reshard (head 1) --------------------
            nc.gpsimd.collective_compute(
                "AllToAll",
                mybir.AluOpType.bypass,
                replica_groups=[list(range(NC))],
                ins=[a2a_in[1].opt()],
                outs=[a2a_out[1].opt()],
            )

            # ---------------- phase C: o_proj ------------------------------
            # k-step order runs all head-0 inner chunks first so the matmuls
            # overlap the head-1 AllToAll
            korder = [2 * r for r in range(NC)] + [2 * r + 1 for r in range(NC)]
            with (
                tc.tile_pool(name="opin", bufs=1) as opinp,
                tc.tile_pool(name="outs", bufs=4) as outsp,
                tc.tile_pool(name="pc", bufs=8, space="PSUM") as pcp,
            ):
                opin = opinp.tile([128, 16, 512], BF16, name="opin")
                for h in range(HPC):
                    for r in range(NC):
                        nc.sync.dma_start(
                            out=opin[:, 2 * r + h, :], in_=a2a_out[h][r]
                        )
                for dc in range(4):
                    wo2 = wo_tiles[dc]
                    pcs = [
                        pcp.tile([128, 512], F32, tag="pc", name=f"pc_{dc}_{ns}")
                        for ns in range(4)
                    ]
                    for ki, k in enumerate(korder):
                        for ns in range(4):
                            nc.tensor.matmul(
                                pcs[ns][:, :],
                                lhsT=(opin[:, k, 128 * ns : 128 * ns + 128]),
                                rhs=(wo2[:, k, :]),
                                start=(ki == 0),
                                stop=(ki == 15),
                            )
                    if dc + 2 < 4:
                        emit_wo_prefetch(dc + 2)
                    for ns in range(4):
                        ost = outsp.tile([128, 512], F32, tag="outs", name=f"os_{dc}_{ns}")
                        nc.vector.tensor_copy(ost[:, :], pcs[ns][:, :])
                        nc.sync.dma_start(
                            out=out[128 * ns : 128 * (ns + 1), 512 * dc : 512 * (dc + 1)],
                            in_=ost[:, :],
                        )
    nc.compile()
    return nc


def _host_prep(x, w_qkv, w_o):
    bf = ml_dtypes.bfloat16
    xT = np.ascontiguousarray(x.reshape(BN, D).T).astype(bf)
    woT = np.ascontiguousarray(np.asarray(w_o).T).astype(bf)

    inv_freq = 1.0 / (ROPE_BASE ** (np.arange(0, HD, 2, dtype=np.float32) / HD))
    ang = np.arange(N, dtype=np.float32)[:, None] * inv_freq[None, :]
    cos_h = np.cos(ang).T.astype(np.float32)      # [64, N]
    sin_h = np.sin(ang).T.astype(np.float32)      # [64, N] (magnitude)
    cos2 = np.concatenate([cos_h, cos_h], axis=0)  # [128, N]
    sin2 = np.concatenate([sin_h, sin_h], axis=0)
    cos_f = np.tile(cos2, (1, B))
    sin_f = np.tile(sin2, (1, B))
    scale = np.float32(1.0 / np.sqrt(HD))
    tabs = np.ascontiguousarray(
        np.stack([cos_f * scale, sin_f * scale, cos_f, sin_f], axis=0)
    ).astype(bf)

    # 128x128 lower-triangular block mask: key row p valid for query col c
    # within a diagonal 128x128 block iff p <= c
    p = np.arange(128)[:, None]
    c = np.arange(128)[None, :]
    tri = (p <= c).astype(bf)

    # rotate-half permutation: (P.T @ x)[i] = -x[i+64] (i<64), x[i-64] (i>=64)
    rotp = np.zeros((128, 128), dtype=np.float32)
    idx = np.arange(64)
    rotp[idx + 64, idx] = -1.0
    rotp[idx, idx + 64] = 1.0
    rotp = rotp.astype(bf)

    in_maps = []
    for core in range(NC):
        h0 = core * HPC
        rq = slice(h0 * HD, (h0 + HPC) * HD)
        rk = slice(INNER + h0 * HD, INNER + (h0 + HPC) * HD)
        rv = slice(2 * INNER + h0 * HD, 2 * INNER + (h0 + HPC) * HD)
        # per-head full-hd blocks: [q_h0, q_h1, k_h0, k_h1]
        wqkT = np.ascontiguousarray(
            np.concatenate([w_qkv[rq], w_qkv[rk]], axis=0).T
        ).astype(bf)
        wvT = np.ascontiguousarray(w_qkv[rv].T).astype(bf)
        in_maps.append(
            dict(xT=xT, wqkT=wqkT, wvT=wvT, woT=woT, tabs=tabs, tri=tri,
                 rotp=rotp)
        )
    return in_maps


def kernel(x, w_qkv, w_o, n_heads=None, head_dim=None, trace=False):
    global LAST_EXEC_NS, LAST_RESULTS
    x = np.asarray(x, dtype=np.float32)
    w_qkv = np.asarray(w_qkv, dtype=np.float32)
    w_o = np.asarray(w_o, dtype=np.float32)

    if "nc" not in _CACHE:
        _CACHE["nc"] = _build_program()
    nc = _CACHE["nc"]

    in_maps = _host_prep(x, w_qkv, w_o)
    res = None
    last_exc = None
    for attempt in range(4):
        try:
            res = bass_utils.run_bass_kernel_spmd(
                nc, in_maps, core_ids=list(range(NC)), trace=trace
            )
            break
        except Exception as e:  # transient compile_and_load / exec flakiness
            last_exc = e
            print(f"kernel attempt {attempt} failed: {e}", file=sys.stderr)
            time.sleep(5)
    if res is None:
        raise last_exc
    LAST_EXEC_NS = res.exec_time_ns
    LAST_RESULTS = res
    shards = [res.results[c]["out"] for c in range(NC)]
    full = np.concatenate(shards, axis=0).reshape(B, N, D).astype(np.float32)
    return full


# revision 3
# speedup vs baseline: 1.0502x; 1.0257x over previous
"""Causal self-attention (B=2, N=2048, D=2048, H=16, hd=128) on 8 Trainium2
NeuronCores — v3.

Tensor-parallel over heads (2 heads/core). v3 = v2 + full-K scores:
  - Per-head weight layout: each 128-row block of wqkT is one head's full
    head_dim, so q/k land on 128 partitions per head and the score matmul
    contracts K=128 in a single instruction (v1/v2 used two K=64 halves,
    which also kept the PE HAM throttled at 1.2 GHz through phase B).
  - RoPE rotate-half done with a PE permutation matmul (P constant) on the
    bf16 copy of the raw projection, then two DVE multiplies + one add.
  - Wide 2-bank exp, PE-accumulated softmax denominators,
    reciprocal_approx_fast, causal narrowing, pipelined score emission,
    double-buffered persist tiles, early woT prefetch (all from v2).
"""

import sys
import time

import ml_dtypes
import numpy as np

sys.path.insert(0, "/opt/trn_rl_repo")

import concourse.bacc as bacc  # noqa: E402
import concourse.bass as bass  # noqa: E402
import concourse.mybir as mybir  # noqa: E402
import concourse.tile as tile  # noqa: E402
from concourse import bass_utils  # noqa: E402

F32 = mybir.dt.float32
BF16 = mybir.dt.bfloat16

B, N, D = 2, 2048, 2048
H, HD = 16, 128
NC = 8
HPC = H // NC          # heads per core
BN = B * N             # 4096
NSH = BN // NC         # output rows per core
INNER = H * HD
ROPE_BASE = 10000.0

_CACHE = {}

LAST_EXEC_NS = None
LAST_RESULTS = None


def _build_program():
    nc = bacc.Bacc(
        "TRN2",
        target_bir_lowering=False,
        debug=False,
        enable_asserts=False,
        num_devices=NC,
    )
    xT = nc.dram_tensor("xT", [D, BN], BF16, kind="ExternalInput").ap()
    wqkT = nc.dram_tensor("wqkT", [D, 4 * HD], BF16, kind="ExternalInput").ap()
    wvT = nc.dram_tensor("wvT", [D, HPC * HD], BF16, kind="ExternalInput").ap()
    woT = nc.dram_tensor("woT", [INNER, D], BF16, kind="ExternalInput").ap()
    tabs = nc.dram_tensor("tabs", [4, HD, BN], BF16, kind="ExternalInput").ap()
    tri = nc.dram_tensor("tri", [128, 128], BF16, kind="ExternalInput").ap()
    rotp = nc.dram_tensor("rotp", [128, 128], BF16, kind="ExternalInput").ap()
    out = nc.dram_tensor("out", [NSH, D], F32, kind="ExternalOutput").ap()
    a2a_in = [
        nc.dram_tensor(f"a2a_in{h}", [NC, 128, 512], BF16).ap()
        for h in range(HPC)
    ]
    a2a_out = [
        nc.dram_tensor(f"a2a_out{h}", [NC, 128, 512], BF16).ap()
        for h in range(HPC)
    ]

    MUL = mybir.AluOpType.mult
    ADD = mybir.AluOpType.add
    SUB = mybir.AluOpType.subtract
    EXP = mybir.ActivationFunctionType.Exp

    with tile.TileContext(nc, num_cores=NC) as tc:
        with (
            tc.tile_pool(name="const", bufs=1) as constp,
            tc.tile_pool(name="wqk", bufs=1) as wqkp,
            tc.tile_pool(name="wv", bufs=1) as wvp,
            tc.tile_pool(name="persist", bufs=2) as persist,
            tc.tile_pool(name="wo", bufs=2) as wop,
        ):
            wqk_sb = wqkp.tile([128, 16, 512], BF16, name="wqk_sb")

            def emit_wqk_chunk(kc):
                nc.sync.dma_start(
                    out=wqk_sb[:, 4 * kc : 4 * kc + 4, :],
                    in_=wqkT.rearrange("(k p) m -> p k m", p=128)[
                        :, 4 * kc : 4 * kc + 4, :
                    ],
                )

            emit_wqk_chunk(0)
            tri_sb = constp.tile([128, 128], BF16, name="tri_sb")
            nc.sync.dma_start(out=tri_sb[:, :], in_=tri[:, :])
            rot_sb = constp.tile([128, 128], BF16, name="rot_sb")
            nc.sync.dma_start(out=rot_sb[:, :], in_=rotp[:, :])
            ones_col = constp.tile([128, 1], BF16, name="ones_col")
            nc.vector.memset(ones_col[:, :], 1.0)
            ones_row = constp.tile([1, 128], F32, name="ones_row")
            nc.vector.memset(ones_row[:, :], 1.0)
            wv_sb = wvp.tile([128, 16, 256], BF16, name="wv_sb")
            wo_tiles = {}

            def emit_wo_prefetch(dc):
                wt = wop.tile([128, 16, 512], BF16, tag="wo2", name=f"wo2_{dc}")
                nc.sync.dma_start(
                    out=wt[:, :, :],
                    in_=woT.rearrange("(k p) d -> p k d", p=128)[
                        :, :, 512 * dc : 512 * (dc + 1)
                    ],
                )
                wo_tiles[dc] = wt

            with (
                tc.tile_pool(name="xt", bufs=3) as xtp,
                tc.tile_pool(name="tab", bufs=2) as tabp,
                tc.tile_pool(name="rope", bufs=2) as ropep,
                tc.tile_pool(name="pt", bufs=3) as ptp,
                tc.tile_pool(name="small", bufs=2) as smallp,
                tc.tile_pool(name="ots", bufs=2) as otsp,
                tc.tile_pool(name="pq", bufs=2, space="PSUM") as pqp,
                tc.tile_pool(name="pov", bufs=2, space="PSUM") as povp,
                tc.tile_pool(name="psmall", bufs=1, space="PSUM") as psmallp,
            ):
                qkT_all = {}
                vT_all = {}
                for b in range(B):
                    qkT_all[b] = persist.tile(
                        [128, 4, N], BF16, tag="qkT", name=f"qkT_b{b}"
                    )
                    vT_all[b] = persist.tile(
                        [128, 16, HPC * HD], BF16, tag="vT", name=f"vT_b{b}"
                    )
                # ---------------- phase A: projection + RoPE (both batches) --
                for b in range(B):
                    qkT_sb = qkT_all[b]
                    vT_sb = vT_all[b]
                    for j in range(4):
                        n0 = b * N + 512 * j
                        first = b == 0 and j == 0
                        xh = []
                        for half in range(2):
                            t = xtp.tile(
                                [128, 8, 512], BF16, tag="xt",
                                name=f"xt_{b}_{j}_{half}",
                            )
                            for q4 in range(2):
                                nc.sync.dma_start(
                                    out=t[:, 4 * q4 : 4 * q4 + 4, :],
                                    in_=xT.rearrange("(k p) n -> p k n", p=128)[
                                        :,
                                        8 * half + 4 * q4 : 8 * half + 4 * q4 + 4,
                                        n0 : n0 + 512,
                                    ],
                                )
                                # interleave remaining weight chunks in the
                                # order the first projection block consumes them
                                if first and 2 * half + q4 < 3:
                                    emit_wqk_chunk(2 * half + q4 + 1)
                            xh.append(t)
                        if first:
                            nc.sync.dma_start(
                                out=wv_sb[:, :, :],
                                in_=wvT.rearrange("(k p) m -> p k m", p=128),
                            )
                        tab4 = tabp.tile(
                            [128, 4, 512], BF16, tag="tab", name=f"tab_{b}_{j}"
                        )
                        nc.sync.dma_start(
                            out=tab4[:, :, :],
                            in_=tabs.rearrange("t p n -> p t n")[
                                :, :, n0 : n0 + 512
                            ],
                        )
                        # qk2[:,0,:] = raw projection (PSUM); qk2[:,1,:] = rotated
                        # (PE permutation of the bf16 copy).  One pending rope
                        # per projection block keeps PE dense.
                        pending_rope = []

                        def emit_rope(mt, qk2):
                            # ci: 0/1 = scaled cos/sin (q heads), 2/3 = cos/sin (k)
                            ci = 0 if mt < 2 else 2
                            raw = ropep.tile(
                                [128, 512], BF16, tag="raw", name=f"raw_{b}_{j}_{mt}"
                            )
                            nc.scalar.copy(raw[:, :], qk2[:, 0, :])
                            nc.tensor.matmul(
                                qk2[:, 1, :], lhsT=rot_sb[:, :], rhs=raw[:, :],
                                start=True, stop=True,
                            )
                            t1 = ropep.tile([128, 512], BF16, tag="t1", name=f"t1_{b}_{j}_{mt}")
                            t2 = ropep.tile([128, 512], BF16, tag="t2", name=f"t2_{b}_{j}_{mt}")
                            nc.vector.tensor_tensor(t1[:, :], raw[:, :], tab4[:, ci, :], MUL)
                            nc.vector.tensor_tensor(t2[:, :], qk2[:, 1, :], tab4[:, ci + 1, :], MUL)
                            nc.vector.tensor_tensor(
                                qkT_sb[:, mt, 512 * j : 512 * (j + 1)], t1[:, :], t2[:, :], ADD
                            )

                        for mt in range(4):
                            qk2 = pqp.tile(
                                [128, 2, 512], F32, tag="st2",
                                name=f"qk2_{b}_{j}_{mt}",
                            )
                            for k in range(16):
                                nc.tensor.matmul(
                                    qk2[:, 0, :],
                                    lhsT=(wqk_sb[:, k, 128 * mt : 128 * mt + 128]),
                                    rhs=(xh[k // 8][:, k % 8, :]),
                                    start=(k == 0),
                                    stop=(k == 15),
                                )
                            if pending_rope:
                                emit_rope(*pending_rope.pop(0))
                            pending_rope.append((mt, qk2))
                        for mt in range(4):
                            pv = povp.tile(
                                [128, 256], F32, tag="ov", name=f"psV_{b}_{j}_{mt}"
                            )
                            for k in range(16):
                                nc.tensor.matmul(
                                    pv[:, :],
                                    lhsT=(xh[k // 8][:, k % 8, 128 * mt : 128 * mt + 128]),
                                    rhs=(wv_sb[:, k, :]),
                                    start=(k == 0),
                                    stop=(k == 15),
                                )
                            if pending_rope:
                                emit_rope(*pending_rope.pop(0))
                            nc.vector.tensor_copy(vT_sb[:, 4 * j + mt, :], pv[:, :])
                        while pending_rope:
                            emit_rope(*pending_rope.pop(0))
                    if b == 0:
                        emit_wo_prefetch(0)
                        emit_wo_prefetch(1)

                # ---------------- phase B: attention (h-major) -------------
                st2_store = {}
                unit_acc = {}

                def emit_scores(b, h, j, tt):
                    qkT_sb = qkT_all[b]
                    st2 = pqp.tile(
                        [128, 2, 512], F32, tag="st2",
                        name=f"st2_{b}_{h}_{j}_{tt}",
                    )
                    f0s = []
                    for u in (0, 1):
                        t = tt + u
                        f0 = max(0, 128 * t - 512 * j)
                        f0s.append(f0)
                        nc.tensor.matmul(
                            st2[:, u, f0:512],
                            lhsT=(qkT_sb[:, 2 + h, 128 * t : 128 * t + 128]),
                            rhs=(qkT_sb[:, h, 512 * j + f0 : 512 * (j + 1)]),
                            start=True,
                            stop=True,
                        )
                    st2_store[(b, h, j, tt)] = (st2, f0s)

                def emit_consume(b, h, j, tt):
                    vT_sb = vT_all[b]
                    nt = 4 * j + 4
                    if tt == 0:
                        unit_acc[(b, h, j)] = (
                            povp.tile([128, 512], F32, tag="ov", name=f"ov_{b}_{h}_{j}"),
                            ropep.tile([128, 512], BF16, tag="rsc", name=f"rsc_{b}_{h}_{j}"),
                        )
                    ov, rs_c = unit_acc[(b, h, j)]
                    st2, f0s = st2_store.pop((b, h, j, tt))
                    ws = min(f0s)
                    pt2 = ptp.tile(
                        [128, 2, 512], BF16, tag="pt", name=f"pt_{b}_{h}_{j}_{tt}"
                    )
                    nc.scalar.activation(
                        pt2[:, :, ws:512], st2[:, :, ws:512], EXP
                    )
                    for u in (0, 1):
                        t = tt + u
                        if t // 4 == j:
                            f0 = f0s[u]
                            nc.vector.tensor_tensor(
                                pt2[:, u, f0 : f0 + 128],
                                pt2[:, u, f0 : f0 + 128],
                                tri_sb[:, :],
                                MUL,
                            )
                    for u in (0, 1):
                        t = tt + u
                        f0 = f0s[u]
                        nc.tensor.matmul(
                            ov[:, f0:512],
                            lhsT=(vT_sb[:, t, 128 * h : 128 * h + 128]),
                            rhs=(pt2[:, u, f0:512]),
                            start=(t == 0),
                            stop=(t == nt - 1),
                        )
                        if t == 0:
                            nc.vector.tensor_copy(rs_c[:, :], pt2[:, 0, :])
                        else:
                            nc.vector.tensor_tensor(
                                rs_c[:, f0:512], rs_c[:, f0:512],
                                pt2[:, u, f0:512], ADD,
                            )

                def make_finalize(b, h, j):
                    ov, rs_c = unit_acc.pop((b, h, j))

                    def fin():
                        rsum = psmallp.tile([1, 512], F32, tag="rsum", name=f"rsum_{b}_{h}_{j}")
                        nc.tensor.matmul(
                            rsum[:, :], lhsT=ones_col[:, :], rhs=rs_c[:, :],
                            start=True, stop=True,
                        )
                        rsum_sb = smallp.tile([1, 512], F32, tag="rsum_sb", name=f"rsb_{b}_{h}_{j}")
                        nc.scalar.copy(rsum_sb[:, :], rsum[:, :])
                        binv = psmallp.tile([128, 512], F32, tag="binv", name=f"binv_{b}_{h}_{j}")
                        nc.tensor.matmul(
                            binv[:, :], lhsT=ones_row[:, :], rhs=rsum_sb[:, :],
                            start=True, stop=True,
                        )
                        rb = smallp.tile([128, 512], F32, tag="rb", name=f"rb_{b}_{h}_{j}")
                        nc.vector.reciprocal_approx_fast(rb[:, :], binv[:, :])
                        ot = otsp.tile([128, 512], BF16, tag="ot", name=f"ot_{b}_{h}_{j}")
                        nc.vector.tensor_tensor(ot[:, :], ov[:, :], rb[:, :], MUL)
                        nc.sync.dma_start(
                            out=a2a_in[h][4 * b + j, :, :], in_=ot[:, :]
                        )
                    return fin

                all_groups = []
                for h in range(HPC):
                    for b in range(B):
                        for j in range(4):
                            for tt in range(0, 4 * j + 4, 2):
                                all_groups.append((b, h, j, tt))
                pending_fin = None
                emit_scores(*all_groups[0])
                for gi, (b, h, j, tt) in enumerate(all_groups):
                    if gi + 1 < len(all_groups):
                        emit_scores(*all_groups[gi + 1])
                    if tt == 0 and pending_fin is not None:
                        pending_fin()
                        pending_fin = None
                    if (b, h, j, tt) == (0, 1, 0, 0):
                        # every h=0 finalize (both batches) has been emitted;
                        # reshard head 0 while head 1 attention still runs
                        nc.gpsimd.collective_compute(
                            "AllToAll",
                            mybir.AluOpType.bypass,
                            replica_groups=[list(range(NC))],
                            ins=[a2a_in[0].opt()],
                            outs=[a2a_out[0].opt()],
                        )
                    emit_consume(b, h, j, tt)
                    if tt == 4 * j + 2:  # last group of unit
                        pending_fin = make_finalize(b, h, j)
                pending_fin()
                pending_fin = None

            # ---------------- AllToAll reshard (head 1) --------------------
            nc.gpsimd.collective_compute(
                "AllToAll",
                mybir.AluOpType.bypass,
                replica_groups=[list(range(NC))],
                ins=[a2a_in[1].opt()],
                outs=[a2a_out[1].opt()],
            )

            # ---------------- phase C: o_proj ------------------------------
            # dc pairs; within a pair, all head-0 K-steps of both dc first so
            # the matmuls overlap the head-1 AllToAll
            with (
                tc.tile_pool(name="opin", bufs=1) as opinp,
                tc.tile_pool(name="outs", bufs=4) as outsp,
                tc.tile_pool(name="pc", bufs=8, space="PSUM") as pcp,
            ):
                opin = opinp.tile([128, 16, 512], BF16, name="opin")
                for h in range(HPC):
                    for r in range(NC):
                        nc.sync.dma_start(
                            out=opin[:, 2 * r + h, :], in_=a2a_out[h][r]
                        )
                for base in (0, 2):
                    pcs_all = {
                        dc: [
                            pcp.tile([128, 512], F32, tag="pc", name=f"pc_{dc}_{ns}")
                            for ns in range(4)
                        ]
                        for dc in (base, base + 1)
                    }
                    for h in range(HPC):
                        for dc in (base, base + 1):
                            wo2 = wo_tiles[dc]
                            for r in range(NC):
                                k = 2 * r + h
                                for ns in range(4):
                                    nc.tensor.matmul(
                                        pcs_all[dc][ns][:, :],
                                        lhsT=(opin[:, k, 128 * ns : 128 * ns + 128]),
                                        rhs=(wo2[:, k, :]),
                                        start=(h == 0 and r == 0),
                                        stop=(h == 1 and r == NC - 1),
                                    )
                    if base == 0:
                        emit_wo_prefetch(2)
                        emit_wo_prefetch(3)
                    for dc in (base, base + 1):
                        for ns in range(4):
                            ost = outsp.tile([128, 512], F32, tag="outs", name=f"os_{dc}_{ns}")
                            nc.vector.tensor_copy(ost[:, :], pcs_all[dc][ns][:, :])
                            nc.sync.dma_start(
                                out=out[128 * ns : 128 * (ns + 1), 512 * dc : 512 * (dc + 1)],
                                in_=ost[:, :],
                            )
    nc.compile()
    return nc


def _host_prep(x, w_qkv, w_o):
    bf = ml_dtypes.bfloat16
    xT = np.ascontiguousarray(x.reshape(BN, D).T).astype(bf)
    woT = np.ascontiguousarray(np.asarray(w_o).T).astype(bf)

    inv_freq = 1.0 / (ROPE_BASE ** (np.arange(0, HD, 2, dtype=np.float32) / HD))
    ang = np.arange(N, dtype=np.float32)[:, None] * inv_freq[None, :]
    cos_h = np.cos(ang).T.astype(np.float32)      # [64, N]
    sin_h = np.sin(ang).T.astype(np.float32)      # [64, N] (magnitude)
    cos2 = np.concatenate([cos_h, cos_h], axis=0)  # [128, N]
    sin2 = np.concatenate([sin_h, sin_h], axis=0)
    cos_f = np.tile(cos2, (1, B))
    sin_f = np.tile(sin2, (1, B))
    scale = np.float32(1.0 / np.sqrt(HD))
    tabs = np.ascontiguousarray(
        np.stack([cos_f * scale, sin_f * scale, cos_f, sin_f], axis=0)
    ).astype(bf)

    # 128x128 lower-triangular block mask: key row p valid for query col c
    # within a diagonal 128x128 block iff p <= c
    p = np.arange(128)[:, None]
    c = np.arange(128)[None, :]
    tri = (p <= c).astype(bf)

    # rotate-half permutation: (P.T @ x)[i] = -x[i+64] (i<64), x[i-64] (i>=64)
    rotp = np.zeros((128, 128), dtype=np.float32)
    idx = np.arange(64)
    rotp[idx + 64, idx] = -1.0
    rotp[idx, idx + 64] = 1.0
    rotp = rotp.astype(bf)

    in_maps = []
    for core in range(NC):
        h0 = core * HPC
        rq = slice(h0 * HD, (h0 + HPC) * HD)
        rk = slice(INNER + h0 * HD, INNER + (h0 + HPC) * HD)
        rv = slice(2 * INNER + h0 * HD, 2 * INNER + (h0 + HPC) * HD)
        # per-head full-hd blocks: [q_h0, q_h1, k_h0, k_h1]
        wqkT = np.ascontiguousarray(
            np.concatenate([w_qkv[rq], w_qkv[rk]], axis=0).T
        ).astype(bf)
        wvT = np.ascontiguousarray(w_qkv[rv].T).astype(bf)
        in_maps.append(
            dict(xT=xT, wqkT=wqkT, wvT=wvT, woT=woT, tabs=tabs, tri=tri,
                 rotp=rotp)
        )
    return in_maps


def kernel(x, w_qkv, w_o, n_heads=None, head_dim=None, trace=False):
    global LAST_EXEC_NS, LAST_RESULTS
    x = np.asarray(x, dtype=np.float32)
    w_qkv = np.asarray(w_qkv, dtype=np.float32)
    w_o = np.asarray(w_o, dtype=np.float32)

    if "nc" not in _CACHE:
        _CACHE["nc"] = _build_program()
    nc = _CACHE["nc"]

    in_maps = _host_prep(x, w_qkv, w_o)
    res = None
    last_exc = None
    for attempt in range(4):
        try:
            res = bass_utils.run_bass_kernel_spmd(
                nc, in_maps, core_ids=list(range(NC)), trace=trace
            )
            break
        except Exception as e:  # transient compile_and_load / exec flakiness
            last_exc = e
            print(f"kernel attempt {attempt} failed: {e}", file=sys.stderr)
            time.sleep(5)
    if res is None:
        raise last_exc
    LAST_EXEC_NS = res.exec_time_ns
    LAST_RESULTS = res
    shards = [res.results[c]["out"] for c in range(NC)]
    full = np.concatenate(shards, axis=0).reshape(B, N, D).astype(np.float32)
    return full
